# revision 1
# baseline (speedup 1.0000x reference)
"""Multi-head attention Trainium2 kernel (8-core SPMD).

Sharding: core c -> batch b = c//4, head-group g = c%4 (4 heads each).
Each core computes partial_out[S, D] = attn(4 heads) @ Wo[rows of its heads].
Host sums the 4 partials per batch (unshard of Wo's contracted input dim) + bo.

Layout strategy (per core, S=2048 D=1024 DK=64, 4 local heads = 2 pairs):
  - host passes x^T [D, S] bf16 so every projection contracts d on partitions.
  - Q/K proj: pair-stacked lhsT=W[d,128] -> QT/KT [2*64, S] psum, copied to
    bf16 "duplicated" per-head tiles (rows 0:64 and 64:128 both hold head h)
    so scores can row-tile two K=64 matmuls concurrently in the PE array.
  - V proj: V4[t, 4*65] bf16, per head 64 cols of V plus a ones column ->
    PV matmul yields [65, s]: rows 0:64 unnormalized out^T, row 64 = rowsum.
  - scores^T[t, s] psum [128, 1024] tiles -> ScalarE exp (scale=1/8 folded)
    -> U bf16; PV accumulates t-outer chasing the exps.
  - normalize via DVE reciprocal + GpSimd partition_broadcast + DVE multiply.
  - Wo: pair-stacked OT2 [128, s] tiles, K=128 matmuls, bf16 partial out
    (host sums the four partials per batch in fp32 and adds bo).
  - emission order is tuned for the in-order engines: both head-pairs
    project in a single x pass (pair 0 in the small psum pool, pair 1 in
    the then-idle big pool), per-head softmax normalization is deferred
    past the next head's first t-iteration, and the Wo epilogue pipelines
    through both PSUM pools.
"""

import os
import sys

import numpy as np

sys.path.insert(0, "/opt/trn_rl_repo")

import ml_dtypes

BF16 = ml_dtypes.bfloat16

_CACHE = {}


def _build_nc(S, D, DK, NH, with_bias=True):
    import concourse.bass as bass
    import concourse.mybir as mybir
    import concourse.tile as tile
    from concourse import bacc

    bf = mybir.dt.bfloat16
    f32 = mybir.dt.float32
    P = 128
    NPAIR = NH // 2
    KT = D // P            # contraction tiles for projections
    TT = S // P            # t-chunks
    SC = S // 512          # 512-wide s-chunks
    SH = S // 1024         # 1024-wide s-halves per t-chunk

    nc = bacc.Bacc("TRN2", target_bir_lowering=False, debug=False)

    xqT = nc.declare_dram_parameter("xqT", [D, S], bf, isOutput=False)
    xkT = nc.declare_dram_parameter("xkT", [D, S], bf, isOutput=False)
    xvT = nc.declare_dram_parameter("xvT", [D, S], bf, isOutput=False)
    wq2 = nc.declare_dram_parameter("wq2", [NPAIR, P, D], bf, isOutput=False)
    wk2 = nc.declare_dram_parameter("wk2", [NPAIR, P, D], bf, isOutput=False)
    wv4 = nc.declare_dram_parameter("wv4", [P, KT * NH * DK], bf, isOutput=False)
    bq2 = nc.declare_dram_parameter("bq2", [NPAIR, 1, P], bf, isOutput=False)
    bk2 = nc.declare_dram_parameter("bk2", [NPAIR, 1, P], bf, isOutput=False)
    bv4 = nc.declare_dram_parameter("bv4", [1, NH * DK], bf, isOutput=False)
    wo2 = nc.declare_dram_parameter("wo2", [NPAIR, P, D], bf, isOutput=False)
    out_d = nc.declare_dram_parameter("out", [S, D], bf, isOutput=True)

    EXP = mybir.ActivationFunctionType.Exp
    scale = 1.0 / np.sqrt(DK)

    with tile.TileContext(nc) as tc:
        with (
            tc.tile_pool(name="consts", bufs=1) as consts,
            tc.tile_pool(name="wp", bufs=1) as wp,
            tc.tile_pool(name="xt", bufs=12) as xt,
            tc.tile_pool(name="qk", bufs=1) as qkp,
            tc.tile_pool(name="vb", bufs=1) as vbp,
            tc.tile_pool(name="up", bufs=22) as up,
            tc.tile_pool(name="ot", bufs=1) as otp,
            tc.tile_pool(name="sm", bufs=4) as smp,
            tc.tile_pool(name="outp", bufs=3) as outp,
            tc.tile_pool(name="psb", bufs=2, space="PSUM") as psb,
            tc.tile_pool(name="pss", bufs=4, space="PSUM") as pss,
        ):
            # constants
            ones_s = consts.tile([1, S], bf, tag="ones_s")
            nc.vector.memset(ones_s[:], 1.0)

            # weights to SBUF (wq first; the rest after the x loads
            # so the first projection's inputs hit DMA earliest)
            wq_sb, wk_sb, bq_sb, bk_sb, wo_sb = [], [], [], [], []
            for p in range(NPAIR):
                wqt = wp.tile([P, D], bf, tag=f"wq{p}")
                nc.sync.dma_start(out=wqt[:], in_=wq2[p])
                wq_sb.append(wqt)
                bqt = wp.tile([1, P], bf, tag=f"bq{p}")
                if with_bias:
                    nc.sync.dma_start(out=bqt[:], in_=bq2[p])
                bq_sb.append(bqt)

            # persistent per-head dup-stacked QT/KT tiles
            QTd = [qkp.tile([P, S], bf, tag=f"qtd{h}", name=f"qtd{h}") for h in range(NH)]
            KTd = [qkp.tile([P, S], bf, tag=f"ktd{h}", name=f"ktd{h}") for h in range(NH)]

            # x tiles: loaded once, resident during their projection
            def load_x(x_dram, nm):
                ts = []
                for k in range(KT):
                    t = xt.tile([P, S], bf, tag="x", name=f"x{nm}{k}")
                    nc.sync.dma_start(out=t[:], in_=x_dram[k * P : (k + 1) * P, :])
                    ts.append(t)
                return ts

            xq_sb = load_x(xqT, "q")
            for p in range(NPAIR):
                wkt = wp.tile([P, D], bf, tag=f"wk{p}")
                nc.sync.dma_start(out=wkt[:], in_=wk2[p])
                wk_sb.append(wkt)
                bkt = wp.tile([1, P], bf, tag=f"bk{p}")
                if with_bias:
                    nc.sync.dma_start(out=bkt[:], in_=bk2[p])
                bk_sb.append(bkt)
            xk_sb = load_x(xkT, "k")

            # Both-pair projection in ONE x pass: pair 0 accumulates in the
            # small psum pool (4x [128,512]), pair 1 in the big pool
            # (2x [128,1024]) which is idle until the first scores.
            def proj_both(x_sb, w0, w1, b0, b1, dst, nm):
                acc = [
                    pss.tile([P, 512], f32, tag="acc", bufs=4, name=f"{nm}ps0_{s}")
                    for s in range(SC)
                ]
                acb = [
                    psb.tile([P, 1024], f32, tag="sc", bufs=2, name=f"{nm}ps1_{j}")
                    for j in range(SH)
                ]
                for k in range(KT):
                    ksl = slice(k * P, (k + 1) * P)
                    for s in range(SC):
                        nc.tensor.matmul(
                            acc[s][:],
                            w0[:, ksl],
                            x_sb[k][:, s * 512 : (s + 1) * 512],
                            start=(k == 0),
                            stop=(not with_bias and k == KT - 1),
                        )
                    for s in range(SC):
                        nc.tensor.matmul(
                            acb[s // 2][:, (s % 2) * 512 : (s % 2 + 1) * 512],
                            w1[:, ksl],
                            x_sb[k][:, s * 512 : (s + 1) * 512],
                            start=(k == 0),
                            stop=(not with_bias and k == KT - 1),
                        )
                for s in range(SC):
                    sl = slice(s * 512, (s + 1) * 512)
                    jsl = slice((s % 2) * 512, (s % 2 + 1) * 512)
                    if with_bias:
                        nc.tensor.matmul(
                            acc[s][:],
                            b0[0:1, :],
                            ones_s[0:1, sl],
                            start=False,
                            stop=True,
                        )
                        nc.tensor.matmul(
                            acb[s // 2][:, jsl],
                            b1[0:1, :],
                            ones_s[0:1, sl],
                            start=False,
                            stop=True,
                        )
                    nc.scalar.copy(dst[0][0:64, sl], acc[s][0:64, :])
                    nc.vector.tensor_copy(dst[1][64:128, sl], acc[s][64:128, :])
                    nc.scalar.copy(dst[2][0:64, sl], acb[s // 2][0:64, jsl])
                    nc.vector.tensor_copy(dst[3][64:128, sl], acb[s // 2][64:128, jsl])

            def dup(dst, h):
                # self-duplicate across partition halves (SBUF->SBUF DMA)
                if h % 2 == 0:
                    nc.sync.dma_start(out=dst[h][64:128, :], in_=dst[h][0:64, :])
                else:
                    nc.sync.dma_start(out=dst[h][0:64, :], in_=dst[h][64:128, :])

            proj_both(xq_sb, wq_sb[0], wq_sb[1], bq_sb[0], bq_sb[1], QTd, "q")
            proj_both(xk_sb, wk_sb[0], wk_sb[1], bk_sb[0], bk_sb[1], KTd, "k")
            dup(KTd, 1)
            dup(QTd, 1)
            wv_sb = wp.tile([P, KT * NH * DK], bf, tag="wv")
            nc.sync.dma_start(out=wv_sb[:], in_=wv4[:])
            bv_sb = wp.tile([1, NH * DK], bf, tag="bv")
            if with_bias:
                nc.sync.dma_start(out=bv_sb[:], in_=bv4[:])
            xv_sb = load_x(xvT, "v")
            dup(KTd, 0)
            dup(QTd, 0)
            dup(KTd, 3)
            dup(QTd, 3)
            dup(KTd, 2)
            dup(QTd, 2)
            for p in range(NPAIR):
                wot = wp.tile([P, D], bf, tag=f"wo{p}")
                nc.sync.dma_start(out=wot[:], in_=wo2[p])
                wo_sb.append(wot)

            # ---- V projection: V4b[t] = [128, NH*65] (V cols + ones col) ----
            # all four 4-t-chunk passes are emitted INSIDE head 1's t-loop
            # (acc pool, transient) so head-1's score matmuls outrank them;
            # head-1's PV emission is lagged past the last pass so its pv
            # allocation follows every vps in the acc-slot FIFO.
            V4b = [None] * TT
            NV = NH * DK

            def v_group(tg):
                vpss = [
                    pss.tile([P, NV], f32, tag="acc", bufs=4, name=f"vps{t}")
                    for t in range(4 * tg, 4 * tg + 4)
                ]
                for k in range(KT):
                    for i, t in enumerate(range(4 * tg, 4 * tg + 4)):
                        nc.tensor.matmul(
                            vpss[i][:],
                            xv_sb[k][:, t * P : (t + 1) * P],
                            wv_sb[:, k * NV : (k + 1) * NV],
                            start=(k == 0),
                            stop=(not with_bias and k == KT - 1),
                        )
                for i, t in enumerate(range(4 * tg, 4 * tg + 4)):
                    if with_bias:
                        nc.tensor.matmul(
                            vpss[i][:],
                            ones_s[0:1, t * P : (t + 1) * P],
                            bv_sb[0:1, :],
                            start=False,
                            stop=True,
                        )
                    vt = vbp.tile(
                        [P, NH * (DK + 1)], bf, tag=f"v4b{t}", name=f"v4b{t}"
                    )
                    nc.vector.tensor_copy(
                        vt.rearrange("p (h e) -> p h e", e=DK + 1)[:, :, 0:DK],
                        vpss[i].rearrange("p (h d) -> p h d", d=DK),
                    )
                    nc.vector.memset(
                        vt.rearrange("p (h e) -> p h e", e=DK + 1)[:, :, DK : DK + 1],
                        1.0,
                    )
                    V4b[t] = vt

            late_v = {
                3 * g + 2: (lambda g=g: v_group(g)) for g in range(TT // 4)
            }
            pv_lag1 = 3 * (TT // 4 - 1) + 3

            # ---- attention per local head ----
            OT2 = [otp.tile([P, S], bf, tag=f"ot{p}", name=f"ot{p}") for p in range(NPAIR)]
            pending_norm = [None]

            def attn_head(h, extra_at=None, pv_lag=0):
                p, r = h // 2, h % 2
                pv = []

                def alloc_pv():
                    pv.extend(
                        pss.tile(
                            [DK + 1, 512], f32, tag="acc", bufs=4, name=f"pv{h}_{s}"
                        )
                        for s in range(SC)
                    )

                if pv_lag == 0:
                    alloc_pv()
                pvq = []
                for t in range(TT):
                    if t == 1 and pending_norm[0] is not None:
                        pending_norm[0]()
                        pending_norm[0] = None
                    if extra_at and t in extra_at:
                        extra_at[t]()
                    if pv_lag and t == pv_lag:
                        alloc_pv()
                        for fn in pvq:
                            fn()
                        pvq = None
                    tsl = slice(t * P, (t + 1) * P)
                    us = []
                    for half in range(SH):
                        sc_t = psb.tile(
                            [P, 1024], f32, tag="sc", bufs=2, name=f"sc{h}_{t}_{half}"
                        )
                        for j in range(2):
                            s0 = half * 1024 + j * 512
                            # head 1's first t-chunks read the direct rows
                            # for both halves (identical data) so the dup
                            # DMA is off the critical path to the first exp
                            if h == 1 and t < 4:
                                jj = 1
                            else:
                                jj = j
                            rp = slice(64 * jj, 64 * (jj + 1))
                            nc.tensor.matmul(
                                sc_t[:, j * 512 : (j + 1) * 512],
                                KTd[h][rp, tsl],
                                QTd[h][rp, s0 : s0 + 512],
                                start=True,
                                stop=True,
                                tile_position=(64 * jj, 0),
                            )
                        ut = up.tile([P, 1024], bf, tag="u", name=f"u{h}_{t}_{half}")
                        nc.scalar.activation(ut[:], sc_t[:], EXP, scale=float(scale))
                        us.append(ut)
                    def emit_pv(t=t, us=us):
                        for s in range(SC):
                            nc.tensor.matmul(
                                pv[s][:],
                                V4b[t][:, h * (DK + 1) : (h + 1) * (DK + 1)],
                                us[s // 2][:, (s % 2) * 512 : (s % 2 + 1) * 512],
                                start=(t == 0),
                                stop=(t == TT - 1),
                            )

                    if pvq is None or not pv_lag:
                        emit_pv()
                    else:
                        pvq.append(emit_pv)
                # normalize: rows 0:64 / row 64 (emitted after the next
                # head's first t-iteration so its scores outrank this chain)
                def make_norm(h=h, p=p, r=r, pv=pv):
                    def norm():
                        if r == 1:
                            ottmp = smp.tile(
                                [64, S], bf, tag="ottmp", bufs=2, name=f"otmp{h}"
                            )
                        for s in range(SC):
                            sl = slice(s * 512, (s + 1) * 512)
                            rsb = smp.tile(
                                [1, 512], f32, tag="r", bufs=4, name=f"r{h}_{s}"
                            )
                            nc.vector.reciprocal(rsb[:], pv[s][64:65, :])
                            bcs = smp.tile(
                                [64, 512], f32, tag="bcs", bufs=4, name=f"bcs{h}_{s}"
                            )
                            nc.gpsimd.partition_broadcast(
                                bcs[:], rsb[0:1, :], channels=64
                            )
                            osb = smp.tile(
                                [64, 512], bf, tag="o", bufs=8, name=f"o{h}_{s}"
                            )
                            nc.vector.tensor_copy(osb[:], pv[s][0:64, :])
                            dst = OT2[p][0:64, sl] if r == 0 else ottmp[:, sl]
                            nc.vector.tensor_mul(dst, osb[:], bcs[:])
                        if r == 1:
                            nc.sync.dma_start(out=OT2[p][64:128, :], in_=ottmp[:, :])
                    return norm

                if pending_norm[0] is not None:
                    pending_norm[0]()
                pending_norm[0] = make_norm()

            attn_head(1, extra_at=late_v, pv_lag=pv_lag1)
            attn_head(0)
            attn_head(3)
            attn_head(2)
            pending_norm[0]()

            # ---- output projection (K = NPAIR*128 accumulated in psum) ----
            for m in range(S // P):
                msl = slice(m * P, (m + 1) * P)
                if m % 2 == 0:
                    # big-pool unit: one [128, 1024] psum tile for both halves
                    wopb = psb.tile([P, D], f32, tag="sc", bufs=2, name=f"wopb{m}")
                    for dj in range(D // 512):
                        for p in range(NPAIR):
                            nc.tensor.matmul(
                                wopb[:, dj * 512 : (dj + 1) * 512],
                                OT2[p][:, msl],
                                wo_sb[p][:, dj * 512 : (dj + 1) * 512],
                                start=(p == 0),
                                stop=(p == NPAIR - 1),
                            )
                    ot_b = outp.tile([P, D], bf, tag="outt", name=f"outt{m}")
                    nc.vector.tensor_copy(ot_b[:], wopb[:])
                    nc.sync.dma_start(out=out_d[msl, :], in_=ot_b[:])
                else:
                    for dj in range(D // 512):
                        wops = pss.tile(
                            [P, 512], f32, tag="acc", bufs=4, name=f"wops{m}_{dj}"
                        )
                        for p in range(NPAIR):
                            nc.tensor.matmul(
                                wops[:],
                                OT2[p][:, msl],
                                wo_sb[p][:, dj * 512 : (dj + 1) * 512],
                                start=(p == 0),
                                stop=(p == NPAIR - 1),
                            )
                        ot_t = outp.tile(
                            [P, 512], bf, tag="outt2", name=f"outt{m}_{dj}"
                        )
                        nc.scalar.copy(ot_t[:], wops[:])
                        nc.sync.dma_start(
                            out=out_d[msl, dj * 512 : (dj + 1) * 512], in_=ot_t[:]
                        )

    nc.finalize()
    return nc


def _prep_core_inputs(query, key, value, Wq, bq, Wk, bk, Wv, bv, Wo, b, g, NH, DK):
    """Host-side shard prep for core (b, g): transpose+cast, pack weights."""
    D = query.shape[2]
    h0 = g * NH
    sl = slice(h0, h0 + NH)
    Wq_g, Wk_g, Wv_g = Wq[sl], Wk[sl], Wv[sl]
    bq_g, bk_g, bv_g = bq[sl], bk[sl], bv[sl]
    NPAIR = NH // 2
    P = 128
    KT = D // P

    def pack_pair(W, bias):
        # [NPAIR, 128, D]: pair p cols = heads (2p, 2p+1) concat; k-major free
        w = np.concatenate(
            [
                np.concatenate([W[2 * p], W[2 * p + 1]], axis=1)[None]
                for p in range(NPAIR)
            ],
            axis=0,
        )  # [NPAIR, D, 128]
        w = w.reshape(NPAIR, KT, P, P).transpose(0, 2, 1, 3).reshape(NPAIR, P, D)
        bb = np.concatenate(
            [
                np.concatenate([bias[2 * p], bias[2 * p + 1]])[None, None]
                for p in range(NPAIR)
            ],
            axis=0,
        )  # [NPAIR, 1, 128]
        return w.astype(BF16), bb.astype(BF16)

    wq2, bq2 = pack_pair(Wq_g, bq_g)
    wk2, bk2 = pack_pair(Wk_g, bk_g)
    wv = np.concatenate([Wv_g[i] for i in range(NH)], axis=1)  # [D, NH*DK]
    NV = NH * DK
    wv4 = wv.reshape(KT, P, NV).transpose(1, 0, 2).reshape(P, KT * NV).astype(BF16)
    bv4 = np.concatenate([bv_g[i] for i in range(NH)])[None].astype(BF16)
    wo2 = (
        Wo[h0 * DK : (h0 + NH) * DK]
        .reshape(NPAIR, P, D)
        .astype(BF16)
    )
    return {
        "xqT": np.ascontiguousarray(query[b].T).astype(BF16),
        "xkT": np.ascontiguousarray(key[b].T).astype(BF16),
        "xvT": np.ascontiguousarray(value[b].T).astype(BF16),
        "wq2": wq2,
        "wk2": wk2,
        "wv4": wv4,
        "bq2": bq2,
        "bk2": bk2,
        "bv4": bv4,
        "wo2": wo2,
    }


def kernel(query, key, value, Wq, bq, Wk, bk, Wv, bv, Wo, bo, _trace=False):
    from concourse.bass_utils import run_bass_kernel_spmd

    query = np.asarray(query, np.float32)
    key = np.asarray(key, np.float32)
    value = np.asarray(value, np.float32)
    B, S, D = query.shape
    H, _, DK = np.asarray(Wq).shape
    NCORE = 8
    GROUPS = NCORE // B
    NH = H // GROUPS

    with_bias = bool(
        np.any(np.asarray(bq)) or np.any(np.asarray(bk)) or np.any(np.asarray(bv))
    )
    ck = ("nc", with_bias)
    if ck not in _CACHE:
        _CACHE[ck] = _build_nc(S, D, DK, NH, with_bias=with_bias)
    nc = _CACHE[ck]

    in_maps = []
    for c in range(NCORE):
        b, g = c // GROUPS, c % GROUPS
        in_maps.append(
            _prep_core_inputs(
                np.asarray(query), np.asarray(key), np.asarray(value),
                np.asarray(Wq), np.asarray(bq), np.asarray(Wk), np.asarray(bk),
                np.asarray(Wv), np.asarray(bv), np.asarray(Wo), b, g, NH, DK,
            )
        )

    res = run_bass_kernel_spmd(nc, in_maps, list(range(NCORE)), trace=_trace)
    out = np.zeros((B, S, D), np.float32)
    for c in range(NCORE):
        out[c // GROUPS] += np.asarray(res.results[c]["out"], np.float32)
    out += np.asarray(bo, np.float32)[None, None, :]
    if _trace:
        _CACHE["last_results"] = res
    return out



# revision 17
# speedup vs baseline: 1.1811x; 1.1811x over previous
"""Multi-head attention Trainium2 kernel (8-core SPMD), v2.

Sharding: core c -> batch b = c//4, head-group g = c%4 (4 local heads = 2
pairs).  Each core computes TWO partial outputs [S, D] (one per head pair,
out = O_pair^T.T @ Wo_pair_rows); the host sums the 8 partials per batch
(4 cores x 2 pairs) in fp32 and adds bo.

Design notes (targets the CoreSim cost model):
  - matmul cost = out_free_size rows; PV is emitted output-[s,dk] oriented:
    lhsT = U (exp scores^T tile, stationary 128x128), moving = V4b [128,65]
    (64 V cols + ones col -> rowsum lands in psum col 64 of each group).
    16640 rows/head instead of 32768.
  - PSUM: 8 banks = scores 2x[128,1024] (4) + PV 3x[128,512] (3) + scratch
    wop [128,512] (1).  PV packs 6/6/4 s-tile groups of 65 fp32 per bank
    using the pending-zero semantics (first matmul in bank start=True, last
    stop=True; later groups' first touch auto-replaces).
  - Act engine runs ONLY the 128 exps ([128,1024] each); all psum->sbuf
    copies go to DVE/Pool, tail copies also Act.
  - O^T for Wo comes from PE transposes into bitcast bf16 views of the
    (just-drained) PV banks; identity built once.
  - DMA order: wq0/wk0, xq&xk first halves interleaved, wq1/wk1/wv, xq h1,
    xk h1, xv h0, xv h1, wo.  First exp ~14.8us; the Act engine (the
    near-critical resource at ~141us of exp work) then streams with only
    small stalls.  A PE warmup matmul at t~0 starts the p-state ramp early.
  - Head 0 is special-cased (chases the x DMAs; its PV spills into head 1's
    iterations); pair-1 Q/K projections and V projection go through the
    scratch bank in the lead-in / early stream.  Head 3 runs its exps
    s-half-0-first so PV bank A (s-tiles 0-5) can be normalized,
    transposed, Wo'd and DMA'd while the s-half-1 exps still stream,
    shrinking the tail to ~10 m-tiles of Wo + out DMA.
"""

import sys

import numpy as np

sys.path.insert(0, "/opt/trn_rl_repo")

import ml_dtypes

BF16 = ml_dtypes.bfloat16

_CACHE = {}

P = 128


def _build_nc(S, D, DK, NH):
    import concourse.bass as bass
    import concourse.mybir as mybir
    import concourse.tile as tile
    import concourse.masks as masks
    from concourse import bacc

    bf = mybir.dt.bfloat16
    f32 = mybir.dt.float32
    NPAIR = NH // 2          # 2
    KT = D // P              # 8 contraction chunks
    TT = S // P              # 16 t-chunks
    NST = S // P             # 16 s-tiles (PV output groups)
    GW = DK + 1              # 65: V cols + ones col
    EXP = mybir.ActivationFunctionType.Exp
    scale = 1.0 / np.sqrt(DK)

    # PV bank group assignment: groups of s-tiles per psum bank
    BANK_GROUPS = [list(range(0, 6)), list(range(6, 12)), list(range(12, 16))]

    nc = bacc.Bacc("TRN2", target_bir_lowering=False, debug=False)

    xqT = nc.declare_dram_parameter("xqT", [D, S], bf, isOutput=False)
    xkT = nc.declare_dram_parameter("xkT", [D, S], bf, isOutput=False)
    xvT = nc.declare_dram_parameter("xvT", [D, S], bf, isOutput=False)
    wq2 = nc.declare_dram_parameter("wq2", [NPAIR, P, D], bf, isOutput=False)
    wk2 = nc.declare_dram_parameter("wk2", [NPAIR, P, D], bf, isOutput=False)
    wv4 = nc.declare_dram_parameter("wv4", [P, KT * NH * DK], bf, isOutput=False)
    wo2 = nc.declare_dram_parameter("wo2", [NPAIR, P, D], bf, isOutput=False)
    out_d = nc.declare_dram_parameter("out", [NPAIR, S, D], bf, isOutput=True)

    with tile.TileContext(nc) as tc:
        with (
            tc.tile_pool(name="consts", bufs=1) as consts,
            tc.tile_pool(name="wp", bufs=1) as wp,
            tc.tile_pool(name="xt", bufs=1) as xt,
            tc.tile_pool(name="qk", bufs=1) as qkp,
            tc.tile_pool(name="vb", bufs=1) as vbp,
            tc.tile_pool(name="up", bufs=18) as up,
            tc.tile_pool(name="onp", bufs=1) as onp,
            tc.tile_pool(name="otp", bufs=1) as otp,
            tc.tile_pool(name="sm", bufs=6) as smp,
            tc.tile_pool(name="outp", bufs=4) as outp,
            tc.tile_pool(name="scp", bufs=2, space="PSUM") as scp,
            tc.tile_pool(name="pvp", bufs=3, space="PSUM") as pvp,
            tc.tile_pool(name="wop", bufs=1, space="PSUM") as wop,
        ):
            # ---- constants ----
            ident = consts.tile([P, P], bf, tag="ident", name="ident")
            masks.make_identity(nc, ident[:])

            # ---- DMA dispatch order (exclusive DMA device => order matters)
            wq_sb = [wp.tile([P, D], bf, tag=f"wq{p}", name=f"wq_sb{p}")
                     for p in range(NPAIR)]
            wk_sb = [wp.tile([P, D], bf, tag=f"wk{p}", name=f"wk_sb{p}")
                     for p in range(NPAIR)]
            nc.sync.dma_start(out=wq_sb[0][:], in_=wq2[0])
            nc.sync.dma_start(out=wk_sb[0][:], in_=wk2[0])

            def x_tiles(nm):
                return [
                    [xt.tile([P, 1024], bf, tag=f"x{nm}{k}_{h}", name=f"x{nm}{k}_{h}")
                     for h in range(2)]
                    for k in range(KT)
                ]

            xq_sb, xk_sb, xv_sb = x_tiles("q"), x_tiles("k"), x_tiles("v")

            def load_x(dram, tiles, k, h):
                nc.sync.dma_start(
                    out=tiles[k][h][:],
                    in_=dram[k * P:(k + 1) * P, h * 1024:(h + 1) * 1024],
                )

            for k in range(KT):          # interleave q/k first halves
                load_x(xqT, xq_sb, k, 0)
                load_x(xkT, xk_sb, k, 0)
            nc.sync.dma_start(out=wq_sb[1][:], in_=wq2[1])
            nc.sync.dma_start(out=wk_sb[1][:], in_=wk2[1])
            wv_sb = wp.tile([P, KT * NH * DK], bf, tag="wv", name="wv_sb")
            nc.sync.dma_start(out=wv_sb[:], in_=wv4[:])
            for k in range(KT):
                load_x(xqT, xq_sb, k, 1)
            for k in range(KT):
                load_x(xkT, xk_sb, k, 1)
            for k in range(KT):
                load_x(xvT, xv_sb, k, 0)
            for k in range(KT):
                load_x(xvT, xv_sb, k, 1)
            wo_sb = [wp.tile([P, D], bf, tag=f"wo{p}", name=f"wo_sb{p}")
                     for p in range(NPAIR)]
            nc.sync.dma_start(out=wo_sb[0][:], in_=wo2[0])
            nc.sync.dma_start(out=wo_sb[1][:], in_=wo2[1])

            # ---- PE p-state warmup: one early matmul starts the ramp clock
            warm = wop.tile([P, 512], f32, tag="wo", name="warm")
            nc.tensor.matmul(warm[:, 0:P], ident[:], ident[:],
                             start=True, stop=True)

            # ---- persistent SBUF tensors ----
            # QT[p][c], KT[p][c]: [128, 512] bf16, c = s-chunk (q) / t-chunk4 (k)
            QT = [[qkp.tile([P, 512], bf, tag=f"qt{p}_{c}", name=f"qt{p}_{c}")
                   for c in range(4)] for p in range(NPAIR)]
            KTt = [[qkp.tile([P, 512], bf, tag=f"kt{p}_{c}", name=f"kt{p}_{c}")
                    for c in range(4)] for p in range(NPAIR)]
            V4b = [vbp.tile([P, NH * GW], bf, tag=f"v4b{t}", name=f"v4b{t}")
                   for t in range(TT)]
            ON = [onp.tile([P, S], bf, tag=f"on{p}", name=f"on{p}")
                  for p in range(NPAIR)]
            OT2 = [[otp.tile([P, P], bf, tag=f"ot2{p}_{m}", name=f"ot2{p}_{m}")
                    for m in range(NST)] for p in range(NPAIR)]

            # ---- helpers ----
            def proj_qk(dst_psum, col, w, xtile, k):
                """Accumulate one k-chunk of a Q/K projection s-chunk."""
                nc.tensor.matmul(
                    dst_psum[:, col * 512:(col + 1) * 512],
                    w[:, k * P:(k + 1) * P],
                    xtile[:, (col % 2) * 512:((col % 2) + 1) * 512],
                    start=(k == 0), stop=(k == KT - 1),
                )

            copy_rr = [0]

            def copy_out(dst, src, eng=None):
                """psum->sbuf copy on a round-robin engine (DVE/Pool)."""
                if eng is None:
                    eng = "dve"
                if eng == "dve":
                    nc.vector.tensor_copy(dst, src)
                elif eng == "pool":
                    nc.gpsimd.tensor_copy(dst, src)
                else:
                    nc.scalar.copy(dst, src)

            def scores_half(h, t, half, sc_t):
                """Two matmuls filling sc_t [128,1024] = scores^T(t-chunk,
                s in [half*1024, half*1024+1024))."""
                p, r = h // 2, h % 2
                rp = slice(64 * r, 64 * (r + 1))
                tp_ = (64 * r, 0) if r else None
                for j in range(2):
                    nc.tensor.matmul(
                        sc_t[:, j * 512:(j + 1) * 512],
                        KTt[p][t // 4][rp, (t % 4) * P:(t % 4 + 1) * P],
                        QT[p][2 * half + j][rp, :],
                        start=True, stop=True, tile_position=tp_,
                    )

            def emit_exp(h, t, half, sc_t):
                ut = up.tile([P, 1024], bf, tag="u", name=f"u{h}_{t}_{half}")
                nc.scalar.activation(ut[:], sc_t[:], EXP, scale=float(scale))
                return ut

            # PV flag tracking: per (head, bank) first/last emission
            def make_pv_flags(order):
                """order: list of (t, g) in emission order -> dict
                (t,g) -> (start, stop) by bank membership."""
                bank_of = {}
                for b, gs in enumerate(BANK_GROUPS):
                    for g in gs:
                        bank_of[g] = b
                first, last = {}, {}
                for (t, g) in order:
                    b = bank_of[g]
                    if b not in first:
                        first[b] = (t, g)
                    last[b] = (t, g)
                return {
                    (t, g): ((t, g) == first[bank_of[g]],
                             (t, g) == last[bank_of[g]])
                    for (t, g) in order
                }

            def pv_mm(h, t, g, pv_tiles, uhalves, flags):
                b = 0 if g < 6 else (1 if g < 12 else 2)
                gi = g - (0, 6, 12)[b]
                st, sp = flags[(t, g)]
                nc.tensor.matmul(
                    pv_tiles[b][:, gi * GW:(gi + 1) * GW],
                    uhalves[(t, g // 8)][:, (g % 8) * P:(g % 8 + 1) * P],
                    V4b[t][:, h * GW:(h + 1) * GW],
                    start=st, stop=sp,
                )

            def emit_norm(h, pv_tiles, banks=(0, 1, 2)):
                """DVE: reciprocal of rowsum col + normalize into ON."""
                p, r = h // 2, h % 2
                for b, gs in enumerate(BANK_GROUPS):
                    if b not in banks:
                        continue
                    n = len(gs)
                    g0 = gs[0]
                    grp = pv_tiles[b][:, 0:n * GW].rearrange(
                        "p (n c) -> p n c", c=GW)
                    rb = smp.tile([P, n], f32, tag="rb", bufs=6,
                                  name=f"rb{h}_{b}")
                    nc.vector.reciprocal(
                        rb[:],
                        grp[:, :, DK:DK + 1].rearrange("p n c -> p (n c)"))
                    onv = ON[p][:].rearrange("p (g x) -> p g x", x=P)[
                        :, g0:g0 + n, r * DK:(r + 1) * DK]
                    nc.vector.tensor_mul(
                        onv,
                        grp[:, :, 0:DK],
                        rb[:].unsqueeze(2).broadcast_to((P, n, DK)),
                    )

            def emit_transpose(p, m, scratch_tile):
                """PE transpose ON[p][:, m*128:(m+1)*128] -> OT2[p][m] via a
                bitcast bf16 view of a drained psum tile."""
                view = scratch_tile[:].bitcast(bf)[:, 0:P]
                nc.tensor.matmul(view, ON[p][:, m * P:(m + 1) * P], ident[:],
                                 is_transpose=True, start=True, stop=True)
                copy_out(OT2[p][m][:], view, eng="dve")

            # V projection: one "pair" = 2 t-chunks through one psum bank
            def v_pair(t0, bank_tile):
                half = t0 // 8
                for k in range(KT):
                    for i in range(2):
                        t = t0 + i
                        nc.tensor.matmul(
                            bank_tile[:, i * 256:(i + 1) * 256],
                            xv_sb[k][half][:, (t - 8 * half) * P:
                                           (t - 8 * half + 1) * P],
                            wv_sb[:, k * NH * DK:(k + 1) * NH * DK],
                            start=(k == 0 and i == 0),
                            stop=(k == KT - 1 and i == 1),
                        )

            def v_copy(t0, bank_tile):
                for i in range(2):
                    t = t0 + i
                    src = bank_tile[:, i * 256:(i + 1) * 256].rearrange(
                        "p (h d) -> p h d", d=DK)
                    dst = V4b[t][:].rearrange("p (h e) -> p h e", e=GW)
                    copy_out(dst[:, :, 0:DK], src)
                    nc.vector.memset(dst[:, :, DK:DK + 1], 1.0)

            # ------------------------------------------------------------
            # Phase A: pair-0 Q/K projections over s-half 0 (chase DMAs)
            # ------------------------------------------------------------
            qp0 = scp.tile([P, 1024], f32, tag="sc", name="qproj0")
            kp0 = scp.tile([P, 1024], f32, tag="sc", name="kproj0")
            for k in range(KT):
                proj_qk(qp0, 0, wq_sb[0], xq_sb[k][0], k)
                proj_qk(qp0, 1, wq_sb[0], xq_sb[k][0], k)
                proj_qk(kp0, 0, wk_sb[0], xk_sb[k][0], k)
                proj_qk(kp0, 1, wk_sb[0], xk_sb[k][0], k)
            copy_out(QT[0][0][:], qp0[:, 0:512], eng="scalar")
            copy_out(QT[0][1][:], qp0[:, 512:1024], eng="scalar")
            copy_out(KTt[0][0][:], kp0[:, 0:512], eng="scalar")
            copy_out(KTt[0][1][:], kp0[:, 512:1024], eng="dve")

            # ------------------------------------------------------------
            # Head 0 with interleaved stuffing.
            # Pair-1 Q/K projections run through the wop bank during the
            # DMA lead-in / early iterations; pair-0 s2/s3 projections
            # chase xq_h1/xk_h1 through the pvp ring.
            # ------------------------------------------------------------
            U = {}

            def p1_use(tensor, c):
                """One pair-1 projection s-chunk through the wop bank."""
                w1 = wq_sb[1] if tensor == 0 else wk_sb[1]
                dst = QT[1][c] if tensor == 0 else KTt[1][c]
                x = xq_sb if tensor == 0 else xk_sb
                pt = wop.tile([P, 512], f32, tag="wo", name=f"p1p{tensor}_{c}")
                for k in range(KT):
                    nc.tensor.matmul(
                        pt[:], w1[:, k * P:(k + 1) * P],
                        x[k][c // 2][:, (c % 2) * 512:((c % 2) + 1) * 512],
                        start=(k == 0), stop=(k == KT - 1))
                copy_out(dst[:], pt[:], eng="dve")

            # pair-1 Q s0/s1 fit in the lead-in (xq half0 resident)
            p1_use(0, 0)
            p1_use(0, 1)

            # pair-0 s2/s3 psum accumulators (pvp ring)
            qp1a = pvp.tile([P, 512], f32, tag="pv", name="qproj2")
            qp1b = pvp.tile([P, 512], f32, tag="pv", name="qproj3")

            h0_exp_order = (
                [(t, 0) for t in range(8)] + [(t, 1) for t in range(8)]
                + [(t, 0) for t in range(8, 16)]
                + [(t, 1) for t in range(8, 16)]
            )
            h0_pv_order = (
                [(t, g) for t in range(8) for g in range(8)]
                + [(t, g) for t in range(8) for g in range(8, 16)]
                + [(t, g) for t in range(8, 16) for g in range(16)]
            )
            h0_flags = make_pv_flags(h0_pv_order)
            pv_tiles_h0 = None
            h0_pv_iter = iter(h0_pv_order)
            h0_left = [len(h0_pv_order)]

            def h0_emit_pv(n):
                for _ in range(n):
                    try:
                        t, g = next(h0_pv_iter)
                    except StopIteration:
                        return
                    pv_mm(0, t, g, pv_tiles_h0,
                          {(t, g // 8): U[(0, t, g // 8)]}, h0_flags)
                    h0_left[0] -= 1

            kp1a = None
            kp1b = None

            idx = 0
            for (t, half) in h0_exp_order:
                sc_t = scp.tile([P, 1024], f32, tag="sc",
                                name=f"sc0_{t}_{half}")
                scores_half(0, t, half, sc_t)
                U[(0, t, half)] = emit_exp(0, t, half, sc_t)

                # pair-1 K s0/s1 through wop, spread to avoid starving Act
                if idx == 5:
                    p1_use(1, 0)
                elif idx == 11:
                    p1_use(1, 1)
                # pair-0 q s2/s3: one k-chunk per iter (chases xq h1)
                if idx < 8:
                    k = idx
                    nc.tensor.matmul(qp1a[:, :],
                                     wq_sb[0][:, k * P:(k + 1) * P],
                                     xq_sb[k][1][:, 0:512],
                                     start=(k == 0), stop=(k == KT - 1))
                    nc.tensor.matmul(qp1b[:, :],
                                     wq_sb[0][:, k * P:(k + 1) * P],
                                     xq_sb[k][1][:, 512:1024],
                                     start=(k == 0), stop=(k == KT - 1))
                    if k == KT - 1:
                        copy_out(QT[0][2][:], qp1a[:, :], eng="dve")
                        copy_out(QT[0][3][:], qp1b[:, :], eng="dve")
                # pair-0 k s2: one k-chunk per iter at idx 6..13
                if idx == 6:
                    kp1a = pvp.tile([P, 512], f32, tag="pv", name="kproj2")
                if 6 <= idx <= 13:
                    k = idx - 6
                    nc.tensor.matmul(kp1a[:, :],
                                     wk_sb[0][:, k * P:(k + 1) * P],
                                     xk_sb[k][1][:, 0:512],
                                     start=(k == 0), stop=(k == KT - 1))
                    if k == KT - 1:
                        copy_out(KTt[0][2][:], kp1a[:, :], eng="dve")
                # pair-0 k s3 at idx 10..17
                if idx == 10:
                    kp1b = pvp.tile([P, 512], f32, tag="pv", name="kproj3")
                if 10 <= idx <= 17:
                    k = idx - 10
                    nc.tensor.matmul(kp1b[:, :],
                                     wk_sb[0][:, k * P:(k + 1) * P],
                                     xk_sb[k][1][:, 512:1024],
                                     start=(k == 0), stop=(k == KT - 1))
                    if k == KT - 1:
                        copy_out(KTt[0][3][:], kp1b[:, :], eng="dve")
                if idx == 19:
                    pv_tiles_h0 = [
                        pvp.tile([P, 512], f32, tag="pv", name=f"pv0_{b}")
                        for b in range(3)
                    ]
                # V projection pairs through wop (split around xv_h1)
                if 20 <= idx <= 23:
                    t0 = (idx - 20) * 2
                    vt = wop.tile([P, 512], f32, tag="wo", name=f"vps{t0}")
                    v_pair(t0, vt)
                    v_copy(t0, vt)
                if 26 <= idx <= 29:
                    t0 = 8 + (idx - 26) * 2
                    vt = wop.tile([P, 512], f32, tag="wo", name=f"vps{t0}")
                    v_pair(t0, vt)
                    v_copy(t0, vt)
                if pv_tiles_h0 is not None and idx >= 22:
                    h0_emit_pv(12)
                idx += 1

            # remaining head-0 PV spills into head 1's early iterations
            pending_norm = [(0, None)]   # placeholder; filled when drained

            # ------------------------------------------------------------
            # Heads 1..3
            # ------------------------------------------------------------
            wo_q = [(m, dj) for m in range(NST) for dj in range(2)]
            wo_pos = [0]
            stage = {}

            def emit_wo_use(pair):
                if wo_pos[0] >= len(wo_q):
                    return
                m, dj = wo_q[wo_pos[0]]
                wo_pos[0] += 1
                wt = wop.tile([P, 512], f32, tag="wo",
                              name=f"wo{pair}_{m}_{dj}")
                nc.tensor.matmul(wt[:], OT2[pair][m][:],
                                 wo_sb[pair][:, dj * 512:(dj + 1) * 512],
                                 start=True, stop=True)
                if m not in stage:
                    stage[m] = outp.tile([P, 1024], bf, tag="stg",
                                         name=f"stg{pair}_{m}")
                copy_out(stage[m][:, dj * 512:(dj + 1) * 512], wt[:],
                         eng="dve")
                if dj == 1:
                    nc.sync.dma_start(
                        out=out_d[pair, m * P:(m + 1) * P, :],
                        in_=stage[m][:])
                    del stage[m]

            tp_queue = []

            # ---------------- head 1: absorbs head-0 PV spill ------------
            h = 1
            flags1 = make_pv_flags([(t, g) for t in range(TT)
                                    for g in range(NST)])
            pv_tiles_h1 = None
            h1_pv_count = [0]

            def h1_emit_pv(upto):
                while h1_pv_count[0] < min(upto, TT) * NST:
                    t = h1_pv_count[0] // NST
                    g = h1_pv_count[0] % NST
                    pv_mm(1, t, g, pv_tiles_h1,
                          {(t, g // 8): U[(1, t, g // 8)]}, flags1)
                    h1_pv_count[0] += 1

            for t in range(TT):
                sc_a = scp.tile([P, 1024], f32, tag="sc", name=f"sc1_{t}_0")
                scores_half(1, t, 0, sc_a)
                U[(1, t, 0)] = emit_exp(1, t, 0, sc_a)
                sc_b = scp.tile([P, 1024], f32, tag="sc", name=f"sc1_{t}_1")
                scores_half(1, t, 1, sc_b)
                U[(1, t, 1)] = emit_exp(1, t, 1, sc_b)

                if t == 1:
                    p1_use(0, 2)
                elif t == 3:
                    p1_use(0, 3)
                elif t == 5:
                    p1_use(1, 2)
                elif t == 7:
                    p1_use(1, 3)
                if t < 8:
                    h0_emit_pv(18)
                if t == 8:
                    h0_emit_pv(10000)    # ensure drained
                    emit_norm(0, pv_tiles_h0)
                    pv_tiles_h1 = [
                        pvp.tile([P, 512], f32, tag="pv", name=f"pv1_{b}")
                        for b in range(3)
                    ]
                if t >= 9:
                    h1_emit_pv(min(3 * (t - 8), t))
            h1_emit_pv(TT)
            pending_norm[0] = (1, pv_tiles_h1)

            # ---------------- head 2: steady state ----------------------
            h = 2
            flags2 = make_pv_flags([(t, g) for t in range(TT)
                                    for g in range(NST)])
            pv_tiles_h2 = None
            for t in range(TT):
                sc_a = scp.tile([P, 1024], f32, tag="sc", name=f"sc2_{t}_0")
                scores_half(2, t, 0, sc_a)
                U[(2, t, 0)] = emit_exp(2, t, 0, sc_a)
                sc_b = scp.tile([P, 1024], f32, tag="sc", name=f"sc2_{t}_1")
                scores_half(2, t, 1, sc_b)
                U[(2, t, 1)] = emit_exp(2, t, 1, sc_b)

                if t == 0:
                    ph, ptiles = pending_norm[0]
                    emit_norm(ph, ptiles)
                    tp_queue.extend((0, m, ptiles) for m in range(NST))
                if t == 1:
                    pv_tiles_h2 = [
                        pvp.tile([P, 512], f32, tag="pv", name=f"pv2_{b}")
                        for b in range(3)
                    ]
                for _ in range(4):
                    if tp_queue:
                        pp, m, ptiles = tp_queue.pop(0)
                        emit_transpose(pp, m, ptiles[m % 3])
                if t >= 1:
                    emit_wo_use(0)
                    if t >= 3 and t % 2 == 1:
                        emit_wo_use(0)
                if t >= 1:
                    for g in range(NST):
                        pv_mm(2, t - 1, g, pv_tiles_h2,
                              {(t - 1, g // 8): U[(2, t - 1, g // 8)]},
                              flags2)
            for g in range(NST):
                pv_mm(2, TT - 1, g, pv_tiles_h2,
                      {(TT - 1, g // 8): U[(2, TT - 1, g // 8)]}, flags2)
            pending_norm[0] = (2, pv_tiles_h2)

            # ---------------- head 3: s0-first + bank-A early flush ------
            h3_exp_order = ([(t, 0) for t in range(TT)]
                            + [(t, 1) for t in range(TT)])
            h3_pv_order = ([(t, g) for t in range(TT) for g in range(8)]
                           + [(t, g) for t in range(TT)
                              for g in range(8, 16)])
            flags3 = make_pv_flags(h3_pv_order)
            pv_tiles_h3 = None
            aflush_q = []    # list of closures for bank-A early flush

            idx2 = 0
            for (t, half) in h3_exp_order:
                sc_t = scp.tile([P, 1024], f32, tag="sc",
                                name=f"sc3_{t}_{half}")
                scores_half(3, t, half, sc_t)
                U[(3, t, half)] = emit_exp(3, t, half, sc_t)

                if idx2 == 0:
                    ph, ptiles = pending_norm[0]
                    emit_norm(ph, ptiles)
                if idx2 == 1:
                    pv_tiles_h3 = [
                        pvp.tile([P, 512], f32, tag="pv", name=f"pv3_{b}")
                        for b in range(3)
                    ]
                if idx2 <= 9:
                    emit_wo_use(0)
                # PV chase: s0 groups during s0 phase, s1 during s1
                if 1 <= idx2 <= 15:
                    tm = idx2 - 1
                    for g in range(8):
                        pv_mm(3, tm, g, pv_tiles_h3,
                              {(tm, 0): U[(3, tm, 0)]}, flags3)
                if idx2 == 16:
                    tm = 15
                    for g in range(8):
                        pv_mm(3, tm, g, pv_tiles_h3,
                              {(tm, 0): U[(3, tm, 0)]}, flags3)
                    # bank A (s-tiles 0..5) is complete: norm + queue flush
                    emit_norm(3, pv_tiles_h3, banks=(0,))
                if 17 <= idx2 <= 31:
                    tm = idx2 - 17
                    for g in range(8, 16):
                        pv_mm(3, tm, g, pv_tiles_h3,
                              {(tm, 1): U[(3, tm, 1)]}, flags3)
                # early flush of m=0..5 during the s1 phase
                if 18 <= idx2 <= 23:
                    m = idx2 - 18
                    emit_transpose(1, m, pv_tiles_h3[0])
                if 20 <= idx2 <= 31:
                    u = idx2 - 20          # 12 wop uses: (m, dj)
                    m, dj = u // 2, u % 2
                    wt = wop.tile([P, 512], f32, tag="wo",
                                  name=f"wo1e_{m}_{dj}")
                    nc.tensor.matmul(wt[:], OT2[1][m][:],
                                     wo_sb[1][:, dj * 512:(dj + 1) * 512],
                                     start=True, stop=True)
                    if m not in stage:
                        stage[m] = outp.tile([P, 1024], bf, tag="stg",
                                             name=f"stg1_{m}")
                    copy_out(stage[m][:, dj * 512:(dj + 1) * 512], wt[:],
                             eng="dve")
                    if dj == 1:
                        nc.sync.dma_start(
                            out=out_d[1, m * P:(m + 1) * P, :],
                            in_=stage[m][:])
                        del stage[m]
                idx2 += 1

            # ------------------------------------------------------------
            # Tail: last PV chunk, norm banks B/C, transposes m6..15,
            # Wo pair-1 for m=6..15 through the sc ring
            # ------------------------------------------------------------
            tm = 15
            for g in range(8, 16):
                pv_mm(3, tm, g, pv_tiles_h3,
                      {(tm, 1): U[(3, tm, 1)]}, flags3)
            emit_norm(3, pv_tiles_h3, banks=(1, 2))
            for m in range(6, NST):
                emit_transpose(1, m, pv_tiles_h3[1 + (m % 2)])

            tail_rr = [0]
            for m in range(6, NST):
                wt = scp.tile([P, 1024], f32, tag="sc", name=f"wo1_{m}")
                for dj in range(2):
                    nc.tensor.matmul(
                        wt[:, dj * 512:(dj + 1) * 512], OT2[1][m][:],
                        wo_sb[1][:, dj * 512:(dj + 1) * 512],
                        start=True, stop=True)
                stg = outp.tile([P, 1024], bf, tag="stg", name=f"stg1t_{m}")
                eng = ("scalar", "dve")[tail_rr[0] % 2]
                tail_rr[0] += 1
                copy_out(stg[:], wt[:], eng=eng)
                nc.sync.dma_start(out=out_d[1, m * P:(m + 1) * P, :],
                                  in_=stg[:])

    nc.finalize()
    return nc


def _prep_core_inputs(query, key, value, Wq, bq, Wk, bk, Wv, bv, Wo,
                      b, g, NH, DK):
    """Host-side shard prep for core (b, g): transpose+cast, pack weights."""
    D = query.shape[2]
    h0 = g * NH
    sl = slice(h0, h0 + NH)
    Wq_g, Wk_g, Wv_g = Wq[sl], Wk[sl], Wv[sl]
    NPAIR = NH // 2
    KT = D // P

    def pack_pair(W):
        # [NPAIR, 128, D]: pair p cols = heads (2p, 2p+1) concat; k-major free
        w = np.concatenate(
            [np.concatenate([W[2 * p], W[2 * p + 1]], axis=1)[None]
             for p in range(NPAIR)], axis=0)          # [NPAIR, D, 128]
        w = w.reshape(NPAIR, KT, P, P).transpose(0, 2, 1, 3).reshape(
            NPAIR, P, D)
        return w.astype(BF16)

    wq2 = pack_pair(Wq_g)
    wk2 = pack_pair(Wk_g)
    wv = np.concatenate([Wv_g[i] for i in range(NH)], axis=1)  # [D, NH*DK]
    NV = NH * DK
    wv4 = wv.reshape(KT, P, NV).transpose(1, 0, 2).reshape(
        P, KT * NV).astype(BF16)
    wo2 = Wo[h0 * DK:(h0 + NH) * DK].reshape(NPAIR, P, D).astype(BF16)
    return {
        "xqT": np.ascontiguousarray(query[b].T).astype(BF16),
        "xkT": np.ascontiguousarray(key[b].T).astype(BF16),
        "xvT": np.ascontiguousarray(value[b].T).astype(BF16),
        "wq2": wq2,
        "wk2": wk2,
        "wv4": wv4,
        "wo2": wo2,
    }


def kernel(query, key, value, Wq, bq, Wk, bk, Wv, bv, Wo, bo, _trace=False):
    from concourse.bass_utils import run_bass_kernel_spmd

    query = np.asarray(query, np.float32)
    key = np.asarray(key, np.float32)
    value = np.asarray(value, np.float32)
    B, S, D = query.shape
    H, _, DK = np.asarray(Wq).shape
    NCORE = 8
    GROUPS = NCORE // B
    NH = H // GROUPS

    # biases are all zero in this problem; verify and fold bo on host
    assert not (np.any(np.asarray(bq)) or np.any(np.asarray(bk))
                or np.any(np.asarray(bv))), "nonzero qkv bias unsupported"

    ck = ("nc",)
    if ck not in _CACHE:
        _CACHE[ck] = _build_nc(S, D, DK, NH)
    nc = _CACHE[ck]

    in_maps = []
    for c in range(NCORE):
        b, g = c // GROUPS, c % GROUPS
        in_maps.append(_prep_core_inputs(
            np.asarray(query), np.asarray(key), np.asarray(value),
            np.asarray(Wq), np.asarray(bq), np.asarray(Wk), np.asarray(bk),
            np.asarray(Wv), np.asarray(bv), np.asarray(Wo), b, g, NH, DK))

    res = run_bass_kernel_spmd(nc, in_maps, list(range(NCORE)), trace=_trace)
    out = np.zeros((B, S, D), np.float32)
    for c in range(NCORE):
        partials = np.asarray(res.results[c]["out"], np.float32)
        out[c // GROUPS] += partials[0] + partials[1]
    out += np.asarray(bo, np.float32)[None, None, :]
    if _trace:
        _CACHE["last_results"] = res
    return out


# revision 18
# speedup vs baseline: 1.1866x; 1.0046x over previous
"""Multi-head attention Trainium2 kernel (8-core SPMD), v2.

Sharding: core c -> batch b = c//4, head-group g = c%4 (4 local heads = 2
pairs).  Each core computes TWO partial outputs [S, D] (one per head pair,
out = O_pair^T.T @ Wo_pair_rows); the host sums the 8 partials per batch
(4 cores x 2 pairs) in fp32 and adds bo.

Design notes (targets the CoreSim cost model):
  - matmul cost = out_free_size rows; PV is emitted output-[s,dk] oriented:
    lhsT = U (exp scores^T tile, stationary 128x128), moving = V4b [128,65]
    (64 V cols + ones col -> rowsum lands in psum col 64 of each group).
    16640 rows/head instead of 32768.
  - PSUM: 8 banks = scores 2x[128,1024] (4) + PV 3x[128,512] (3) + scratch
    wop [128,512] (1).  PV packs 6/6/4 s-tile groups of 65 fp32 per bank
    using the pending-zero semantics (first matmul in bank start=True, last
    stop=True; later groups' first touch auto-replaces).
  - Act engine runs ONLY the 128 exps ([128,1024] each); all psum->sbuf
    copies go to DVE/Pool, tail copies also Act.
  - O^T for Wo comes from PE transposes into bitcast bf16 views of the
    (just-drained) PV banks; identity built once.
  - DMA order: wq0/wk0, xq&xk first halves interleaved, wq1/wk1/wv, xq h1,
    xk h1, xv h0, xv h1, wo.  First exp ~14.8us; the Act engine (the
    near-critical resource at ~141us of exp work) then streams with only
    small stalls.  A PE warmup matmul at t~0 starts the p-state ramp early.
  - Head 0 is special-cased (chases the x DMAs; its PV spills into head 1's
    iterations); pair-1 Q/K projections and V projection go through the
    scratch bank in the lead-in / early stream.  Head 3 runs its exps
    s-half-0-first so PV bank A (s-tiles 0-5) can be normalized,
    transposed, Wo'd and DMA'd while the s-half-1 exps still stream,
    shrinking the tail to ~10 m-tiles of Wo + out DMA.
"""

import sys

import numpy as np

sys.path.insert(0, "/opt/trn_rl_repo")

import ml_dtypes

BF16 = ml_dtypes.bfloat16

_CACHE = {}

P = 128


def _build_nc(S, D, DK, NH):
    import concourse.bass as bass
    import concourse.mybir as mybir
    import concourse.tile as tile
    import concourse.masks as masks
    from concourse import bacc

    bf = mybir.dt.bfloat16
    f32 = mybir.dt.float32
    NPAIR = NH // 2          # 2
    KT = D // P              # 8 contraction chunks
    TT = S // P              # 16 t-chunks
    NST = S // P             # 16 s-tiles (PV output groups)
    GW = DK + 1              # 65: V cols + ones col
    EXP = mybir.ActivationFunctionType.Exp
    scale = 1.0 / np.sqrt(DK)

    # PV bank group assignment: groups of s-tiles per psum bank
    BANK_GROUPS = [list(range(0, 6)), list(range(6, 12)), list(range(12, 16))]

    nc = bacc.Bacc("TRN2", target_bir_lowering=False, debug=False)

    xqT = nc.declare_dram_parameter("xqT", [D, S], bf, isOutput=False)
    xkT = nc.declare_dram_parameter("xkT", [D, S], bf, isOutput=False)
    xvT = nc.declare_dram_parameter("xvT", [D, S], bf, isOutput=False)
    wq2 = nc.declare_dram_parameter("wq2", [NPAIR, P, D], bf, isOutput=False)
    wk2 = nc.declare_dram_parameter("wk2", [NPAIR, P, D], bf, isOutput=False)
    wv4 = nc.declare_dram_parameter("wv4", [P, KT * NH * DK], bf, isOutput=False)
    wo2 = nc.declare_dram_parameter("wo2", [NPAIR, P, D], bf, isOutput=False)
    out_d = nc.declare_dram_parameter("out", [NPAIR, S, D], bf, isOutput=True)

    with tile.TileContext(nc) as tc:
        with (
            tc.tile_pool(name="consts", bufs=1) as consts,
            tc.tile_pool(name="wp", bufs=1) as wp,
            tc.tile_pool(name="xt", bufs=1) as xt,
            tc.tile_pool(name="qk", bufs=1) as qkp,
            tc.tile_pool(name="vb", bufs=1) as vbp,
            tc.tile_pool(name="up", bufs=21) as up,
            tc.tile_pool(name="onp", bufs=1) as onp,
            tc.tile_pool(name="otp", bufs=1) as otp,
            tc.tile_pool(name="sm", bufs=6) as smp,
            tc.tile_pool(name="outp", bufs=4) as outp,
            tc.tile_pool(name="scp", bufs=2, space="PSUM") as scp,
            tc.tile_pool(name="pvp", bufs=3, space="PSUM") as pvp,
            tc.tile_pool(name="wop", bufs=1, space="PSUM") as wop,
        ):
            # ---- constants ----
            ident = consts.tile([P, P], bf, tag="ident", name="ident")
            masks.make_identity(nc, ident[:])

            # ---- DMA dispatch order (exclusive DMA device => order matters)
            wq_sb = [wp.tile([P, D], bf, tag=f"wq{p}", name=f"wq_sb{p}")
                     for p in range(NPAIR)]
            wk_sb = [wp.tile([P, D], bf, tag=f"wk{p}", name=f"wk_sb{p}")
                     for p in range(NPAIR)]
            nc.sync.dma_start(out=wq_sb[0][:], in_=wq2[0])
            nc.sync.dma_start(out=wk_sb[0][:], in_=wk2[0])

            def x_tiles(nm):
                return [
                    [xt.tile([P, 1024], bf, tag=f"x{nm}{k}_{h}", name=f"x{nm}{k}_{h}")
                     for h in range(2)]
                    for k in range(KT)
                ]

            xq_sb, xk_sb, xv_sb = x_tiles("q"), x_tiles("k"), x_tiles("v")

            def load_x(dram, tiles, k, h):
                nc.sync.dma_start(
                    out=tiles[k][h][:],
                    in_=dram[k * P:(k + 1) * P, h * 1024:(h + 1) * 1024],
                )

            for k in range(KT):          # interleave q/k first halves
                load_x(xqT, xq_sb, k, 0)
                load_x(xkT, xk_sb, k, 0)
            nc.sync.dma_start(out=wq_sb[1][:], in_=wq2[1])
            nc.sync.dma_start(out=wk_sb[1][:], in_=wk2[1])
            wv_sb = wp.tile([P, KT * NH * DK], bf, tag="wv", name="wv_sb")
            nc.sync.dma_start(out=wv_sb[:], in_=wv4[:])
            for k in range(KT):
                load_x(xqT, xq_sb, k, 1)
            for k in range(KT):
                load_x(xkT, xk_sb, k, 1)
            for k in range(KT):
                load_x(xvT, xv_sb, k, 0)
            for k in range(KT):
                load_x(xvT, xv_sb, k, 1)
            wo_sb = [wp.tile([P, D], bf, tag=f"wo{p}", name=f"wo_sb{p}")
                     for p in range(NPAIR)]
            nc.sync.dma_start(out=wo_sb[0][:], in_=wo2[0])
            nc.sync.dma_start(out=wo_sb[1][:], in_=wo2[1])

            # ---- PE p-state warmup: one early matmul starts the ramp clock
            warm = wop.tile([P, 512], f32, tag="wo", name="warm")
            nc.tensor.matmul(warm[:, 0:P], ident[:], ident[:],
                             start=True, stop=True)

            # ---- persistent SBUF tensors ----
            # QT[p][c], KT[p][c]: [128, 512] bf16, c = s-chunk (q) / t-chunk4 (k)
            QT = [[qkp.tile([P, 512], bf, tag=f"qt{p}_{c}", name=f"qt{p}_{c}")
                   for c in range(4)] for p in range(NPAIR)]
            KTt = [[qkp.tile([P, 512], bf, tag=f"kt{p}_{c}", name=f"kt{p}_{c}")
                    for c in range(4)] for p in range(NPAIR)]
            V4b = [vbp.tile([P, NH * GW], bf, tag=f"v4b{t}", name=f"v4b{t}")
                   for t in range(TT)]
            ON = [onp.tile([P, S], bf, tag=f"on{p}", name=f"on{p}")
                  for p in range(NPAIR)]
            OT2 = [[otp.tile([P, P], bf, tag=f"ot2{p}_{m}", name=f"ot2{p}_{m}")
                    for m in range(NST)] for p in range(NPAIR)]

            # ---- helpers ----
            def proj_qk(dst_psum, col, w, xtile, k):
                """Accumulate one k-chunk of a Q/K projection s-chunk."""
                nc.tensor.matmul(
                    dst_psum[:, col * 512:(col + 1) * 512],
                    w[:, k * P:(k + 1) * P],
                    xtile[:, (col % 2) * 512:((col % 2) + 1) * 512],
                    start=(k == 0), stop=(k == KT - 1),
                )

            copy_rr = [0]

            def copy_out(dst, src, eng=None):
                """psum->sbuf copy on a round-robin engine (DVE/Pool)."""
                if eng is None:
                    eng = "dve"
                if eng == "dve":
                    nc.vector.tensor_copy(dst, src)
                elif eng == "pool":
                    nc.gpsimd.tensor_copy(dst, src)
                else:
                    nc.scalar.copy(dst, src)

            def scores_half(h, t, half, sc_t):
                """Two matmuls filling sc_t [128,1024] = scores^T(t-chunk,
                s in [half*1024, half*1024+1024))."""
                p, r = h // 2, h % 2
                rp = slice(64 * r, 64 * (r + 1))
                tp_ = (64 * r, 0) if r else None
                for j in range(2):
                    nc.tensor.matmul(
                        sc_t[:, j * 512:(j + 1) * 512],
                        KTt[p][t // 4][rp, (t % 4) * P:(t % 4 + 1) * P],
                        QT[p][2 * half + j][rp, :],
                        start=True, stop=True, tile_position=tp_,
                    )

            def emit_exp(h, t, half, sc_t):
                ut = up.tile([P, 1024], bf, tag="u", name=f"u{h}_{t}_{half}")
                nc.scalar.activation(ut[:], sc_t[:], EXP, scale=float(scale))
                return ut

            # PV flag tracking: per (head, bank) first/last emission
            def make_pv_flags(order):
                """order: list of (t, g) in emission order -> dict
                (t,g) -> (start, stop) by bank membership."""
                bank_of = {}
                for b, gs in enumerate(BANK_GROUPS):
                    for g in gs:
                        bank_of[g] = b
                first, last = {}, {}
                for (t, g) in order:
                    b = bank_of[g]
                    if b not in first:
                        first[b] = (t, g)
                    last[b] = (t, g)
                return {
                    (t, g): ((t, g) == first[bank_of[g]],
                             (t, g) == last[bank_of[g]])
                    for (t, g) in order
                }

            def pv_mm(h, t, g, pv_tiles, uhalves, flags):
                b = 0 if g < 6 else (1 if g < 12 else 2)
                gi = g - (0, 6, 12)[b]
                st, sp = flags[(t, g)]
                nc.tensor.matmul(
                    pv_tiles[b][:, gi * GW:(gi + 1) * GW],
                    uhalves[(t, g // 8)][:, (g % 8) * P:(g % 8 + 1) * P],
                    V4b[t][:, h * GW:(h + 1) * GW],
                    start=st, stop=sp,
                )

            def emit_norm(h, pv_tiles, banks=(0, 1, 2)):
                """DVE: reciprocal of rowsum col + normalize into ON."""
                p, r = h // 2, h % 2
                for b, gs in enumerate(BANK_GROUPS):
                    if b not in banks:
                        continue
                    n = len(gs)
                    g0 = gs[0]
                    grp = pv_tiles[b][:, 0:n * GW].rearrange(
                        "p (n c) -> p n c", c=GW)
                    rb = smp.tile([P, n], f32, tag="rb", bufs=6,
                                  name=f"rb{h}_{b}")
                    nc.vector.reciprocal(
                        rb[:],
                        grp[:, :, DK:DK + 1].rearrange("p n c -> p (n c)"))
                    onv = ON[p][:].rearrange("p (g x) -> p g x", x=P)[
                        :, g0:g0 + n, r * DK:(r + 1) * DK]
                    nc.vector.tensor_mul(
                        onv,
                        grp[:, :, 0:DK],
                        rb[:].unsqueeze(2).broadcast_to((P, n, DK)),
                    )

            def emit_transpose(p, m, scratch_tile):
                """PE transpose ON[p][:, m*128:(m+1)*128] -> OT2[p][m] via a
                bitcast bf16 view of a drained psum tile."""
                view = scratch_tile[:].bitcast(bf)[:, 0:P]
                nc.tensor.matmul(view, ON[p][:, m * P:(m + 1) * P], ident[:],
                                 is_transpose=True, start=True, stop=True)
                copy_out(OT2[p][m][:], view, eng="dve")

            # V projection: one "pair" = 2 t-chunks through one psum bank
            def v_pair(t0, bank_tile):
                half = t0 // 8
                for k in range(KT):
                    for i in range(2):
                        t = t0 + i
                        nc.tensor.matmul(
                            bank_tile[:, i * 256:(i + 1) * 256],
                            xv_sb[k][half][:, (t - 8 * half) * P:
                                           (t - 8 * half + 1) * P],
                            wv_sb[:, k * NH * DK:(k + 1) * NH * DK],
                            start=(k == 0 and i == 0),
                            stop=(k == KT - 1 and i == 1),
                        )

            def v_copy(t0, bank_tile):
                for i in range(2):
                    t = t0 + i
                    src = bank_tile[:, i * 256:(i + 1) * 256].rearrange(
                        "p (h d) -> p h d", d=DK)
                    dst = V4b[t][:].rearrange("p (h e) -> p h e", e=GW)
                    copy_out(dst[:, :, 0:DK], src)
                    nc.vector.memset(dst[:, :, DK:DK + 1], 1.0)

            # ------------------------------------------------------------
            # Phase A: pair-0 Q/K projections over s-half 0 (chase DMAs)
            # ------------------------------------------------------------
            qp0 = scp.tile([P, 1024], f32, tag="sc", name="qproj0")
            kp0 = scp.tile([P, 1024], f32, tag="sc", name="kproj0")
            for k in range(KT):
                proj_qk(qp0, 0, wq_sb[0], xq_sb[k][0], k)
                proj_qk(qp0, 1, wq_sb[0], xq_sb[k][0], k)
                proj_qk(kp0, 0, wk_sb[0], xk_sb[k][0], k)
                proj_qk(kp0, 1, wk_sb[0], xk_sb[k][0], k)
            copy_out(QT[0][0][:], qp0[:, 0:512], eng="scalar")
            copy_out(QT[0][1][:], qp0[:, 512:1024], eng="scalar")
            copy_out(KTt[0][0][:], kp0[:, 0:512], eng="scalar")
            copy_out(KTt[0][1][:], kp0[:, 512:1024], eng="dve")

            # ------------------------------------------------------------
            # Head 0 with interleaved stuffing.
            # Pair-1 Q/K projections run through the wop bank during the
            # DMA lead-in / early iterations; pair-0 s2/s3 projections
            # chase xq_h1/xk_h1 through the pvp ring.
            # ------------------------------------------------------------
            U = {}

            def p1_use(tensor, c):
                """One pair-1 projection s-chunk through the wop bank."""
                w1 = wq_sb[1] if tensor == 0 else wk_sb[1]
                dst = QT[1][c] if tensor == 0 else KTt[1][c]
                x = xq_sb if tensor == 0 else xk_sb
                pt = wop.tile([P, 512], f32, tag="wo", name=f"p1p{tensor}_{c}")
                for k in range(KT):
                    nc.tensor.matmul(
                        pt[:], w1[:, k * P:(k + 1) * P],
                        x[k][c // 2][:, (c % 2) * 512:((c % 2) + 1) * 512],
                        start=(k == 0), stop=(k == KT - 1))
                copy_out(dst[:], pt[:], eng="dve")

            # pair-1 Q s0/s1 fit in the lead-in (xq half0 resident)
            p1_use(0, 0)
            p1_use(0, 1)

            # pair-0 s2/s3 psum accumulators (pvp ring)
            qp1a = pvp.tile([P, 512], f32, tag="pv", name="qproj2")
            qp1b = pvp.tile([P, 512], f32, tag="pv", name="qproj3")

            h0_exp_order = (
                [(t, 0) for t in range(8)] + [(t, 1) for t in range(8)]
                + [(t, 0) for t in range(8, 16)]
                + [(t, 1) for t in range(8, 16)]
            )
            h0_pv_order = (
                [(t, g) for t in range(8) for g in range(8)]
                + [(t, g) for t in range(8) for g in range(8, 16)]
                + [(t, g) for t in range(8, 16) for g in range(16)]
            )
            h0_flags = make_pv_flags(h0_pv_order)
            pv_tiles_h0 = None
            h0_pv_iter = iter(h0_pv_order)
            h0_left = [len(h0_pv_order)]

            def h0_emit_pv(n):
                for _ in range(n):
                    try:
                        t, g = next(h0_pv_iter)
                    except StopIteration:
                        return
                    pv_mm(0, t, g, pv_tiles_h0,
                          {(t, g // 8): U[(0, t, g // 8)]}, h0_flags)
                    h0_left[0] -= 1

            kp1a = None
            kp1b = None

            idx = 0
            for (t, half) in h0_exp_order:
                sc_t = scp.tile([P, 1024], f32, tag="sc",
                                name=f"sc0_{t}_{half}")
                scores_half(0, t, half, sc_t)
                U[(0, t, half)] = emit_exp(0, t, half, sc_t)

                # pair-1 K s0/s1 through wop, spread to avoid starving Act
                if idx == 5:
                    p1_use(1, 0)
                elif idx == 11:
                    p1_use(1, 1)
                # pair-0 q s2/s3: one k-chunk per iter (chases xq h1)
                if idx < 8:
                    k = idx
                    nc.tensor.matmul(qp1a[:, :],
                                     wq_sb[0][:, k * P:(k + 1) * P],
                                     xq_sb[k][1][:, 0:512],
                                     start=(k == 0), stop=(k == KT - 1))
                    nc.tensor.matmul(qp1b[:, :],
                                     wq_sb[0][:, k * P:(k + 1) * P],
                                     xq_sb[k][1][:, 512:1024],
                                     start=(k == 0), stop=(k == KT - 1))
                    if k == KT - 1:
                        copy_out(QT[0][2][:], qp1a[:, :], eng="dve")
                        copy_out(QT[0][3][:], qp1b[:, :], eng="dve")
                # pair-0 k s2: one k-chunk per iter at idx 6..13
                if idx == 6:
                    kp1a = pvp.tile([P, 512], f32, tag="pv", name="kproj2")
                if 6 <= idx <= 13:
                    k = idx - 6
                    nc.tensor.matmul(kp1a[:, :],
                                     wk_sb[0][:, k * P:(k + 1) * P],
                                     xk_sb[k][1][:, 0:512],
                                     start=(k == 0), stop=(k == KT - 1))
                    if k == KT - 1:
                        copy_out(KTt[0][2][:], kp1a[:, :], eng="dve")
                # pair-0 k s3 at idx 10..17
                if idx == 10:
                    kp1b = pvp.tile([P, 512], f32, tag="pv", name="kproj3")
                if 10 <= idx <= 17:
                    k = idx - 10
                    nc.tensor.matmul(kp1b[:, :],
                                     wk_sb[0][:, k * P:(k + 1) * P],
                                     xk_sb[k][1][:, 512:1024],
                                     start=(k == 0), stop=(k == KT - 1))
                    if k == KT - 1:
                        copy_out(KTt[0][3][:], kp1b[:, :], eng="dve")
                # V projection, two lanes: t4-7 borrow the pvp ring
                # before the PV banks are claimed; the rest go through wop.
                if idx in (18, 19):
                    t0 = 4 + (idx - 18) * 2
                    vtp = pvp.tile([P, 512], f32, tag="pv",
                                   name=f"vps{t0}p")
                    v_pair(t0, vtp)
                    v_copy(t0, vtp)
                if idx == 20:
                    pv_tiles_h0 = [
                        pvp.tile([P, 512], f32, tag="pv", name=f"pv0_{b}")
                        for b in range(3)
                    ]
                if idx in (19, 20):
                    t0 = (idx - 19) * 2
                    vt = wop.tile([P, 512], f32, tag="wo", name=f"vps{t0}")
                    v_pair(t0, vt)
                    v_copy(t0, vt)
                if 25 <= idx <= 28:
                    t0 = 8 + (idx - 25) * 2
                    vt = wop.tile([P, 512], f32, tag="wo", name=f"vps{t0}")
                    v_pair(t0, vt)
                    v_copy(t0, vt)
                if pv_tiles_h0 is not None and idx >= 22:
                    h0_emit_pv(12)
                idx += 1

            # remaining head-0 PV spills into head 1's early iterations
            pending_norm = [(0, None)]   # placeholder; filled when drained

            # ------------------------------------------------------------
            # Heads 1..3
            # ------------------------------------------------------------
            wo_q = [(m, dj) for m in range(NST) for dj in range(2)]
            wo_pos = [0]
            stage = {}

            def emit_wo_use(pair):
                if wo_pos[0] >= len(wo_q):
                    return
                m, dj = wo_q[wo_pos[0]]
                wo_pos[0] += 1
                wt = wop.tile([P, 512], f32, tag="wo",
                              name=f"wo{pair}_{m}_{dj}")
                nc.tensor.matmul(wt[:], OT2[pair][m][:],
                                 wo_sb[pair][:, dj * 512:(dj + 1) * 512],
                                 start=True, stop=True)
                if m not in stage:
                    stage[m] = outp.tile([P, 1024], bf, tag="stg",
                                         name=f"stg{pair}_{m}")
                copy_out(stage[m][:, dj * 512:(dj + 1) * 512], wt[:],
                         eng="dve")
                if dj == 1:
                    nc.sync.dma_start(
                        out=out_d[pair, m * P:(m + 1) * P, :],
                        in_=stage[m][:])
                    del stage[m]

            tp_queue = []

            # ---------------- head 1: absorbs head-0 PV spill ------------
            h = 1
            flags1 = make_pv_flags([(t, g) for t in range(TT)
                                    for g in range(NST)])
            pv_tiles_h1 = None
            h1_pv_count = [0]

            def h1_emit_pv(upto):
                while h1_pv_count[0] < min(upto, TT) * NST:
                    t = h1_pv_count[0] // NST
                    g = h1_pv_count[0] % NST
                    pv_mm(1, t, g, pv_tiles_h1,
                          {(t, g // 8): U[(1, t, g // 8)]}, flags1)
                    h1_pv_count[0] += 1

            for t in range(TT):
                sc_a = scp.tile([P, 1024], f32, tag="sc", name=f"sc1_{t}_0")
                scores_half(1, t, 0, sc_a)
                U[(1, t, 0)] = emit_exp(1, t, 0, sc_a)
                sc_b = scp.tile([P, 1024], f32, tag="sc", name=f"sc1_{t}_1")
                scores_half(1, t, 1, sc_b)
                U[(1, t, 1)] = emit_exp(1, t, 1, sc_b)

                if t == 1:
                    p1_use(0, 2)
                elif t == 3:
                    p1_use(0, 3)
                elif t == 5:
                    p1_use(1, 2)
                elif t == 7:
                    p1_use(1, 3)
                if t < 8:
                    h0_emit_pv(18)
                if t == 8:
                    h0_emit_pv(10000)    # ensure drained
                    emit_norm(0, pv_tiles_h0)
                    pv_tiles_h1 = [
                        pvp.tile([P, 512], f32, tag="pv", name=f"pv1_{b}")
                        for b in range(3)
                    ]
                if t >= 9:
                    h1_emit_pv(min(3 * (t - 8), t))
            h1_emit_pv(TT)
            pending_norm[0] = (1, pv_tiles_h1)

            # ---------------- head 2: steady state ----------------------
            h = 2
            flags2 = make_pv_flags([(t, g) for t in range(TT)
                                    for g in range(NST)])
            pv_tiles_h2 = None
            for t in range(TT):
                sc_a = scp.tile([P, 1024], f32, tag="sc", name=f"sc2_{t}_0")
                scores_half(2, t, 0, sc_a)
                U[(2, t, 0)] = emit_exp(2, t, 0, sc_a)
                sc_b = scp.tile([P, 1024], f32, tag="sc", name=f"sc2_{t}_1")
                scores_half(2, t, 1, sc_b)
                U[(2, t, 1)] = emit_exp(2, t, 1, sc_b)

                if t == 0:
                    ph, ptiles = pending_norm[0]
                    emit_norm(ph, ptiles)
                    tp_queue.extend((0, m, ptiles) for m in range(NST))
                if t == 1:
                    pv_tiles_h2 = [
                        pvp.tile([P, 512], f32, tag="pv", name=f"pv2_{b}")
                        for b in range(3)
                    ]
                for _ in range(4):
                    if tp_queue:
                        pp, m, ptiles = tp_queue.pop(0)
                        emit_transpose(pp, m, ptiles[m % 3])
                if t >= 1:
                    emit_wo_use(0)
                    if t >= 3 and t % 2 == 1:
                        emit_wo_use(0)
                if t >= 1:
                    for g in range(NST):
                        pv_mm(2, t - 1, g, pv_tiles_h2,
                              {(t - 1, g // 8): U[(2, t - 1, g // 8)]},
                              flags2)
            for g in range(NST):
                pv_mm(2, TT - 1, g, pv_tiles_h2,
                      {(TT - 1, g // 8): U[(2, TT - 1, g // 8)]}, flags2)
            pending_norm[0] = (2, pv_tiles_h2)

            # ---------------- head 3: s0-first + bank-A early flush ------
            h3_exp_order = ([(t, 0) for t in range(TT)]
                            + [(t, 1) for t in range(TT)])
            h3_pv_order = ([(t, g) for t in range(TT) for g in range(8)]
                           + [(t, g) for t in range(TT)
                              for g in range(8, 16)])
            flags3 = make_pv_flags(h3_pv_order)
            pv_tiles_h3 = None
            aflush_q = []    # list of closures for bank-A early flush

            idx2 = 0
            for (t, half) in h3_exp_order:
                sc_t = scp.tile([P, 1024], f32, tag="sc",
                                name=f"sc3_{t}_{half}")
                scores_half(3, t, half, sc_t)
                U[(3, t, half)] = emit_exp(3, t, half, sc_t)

                if idx2 == 0:
                    ph, ptiles = pending_norm[0]
                    emit_norm(ph, ptiles)
                if idx2 == 1:
                    pv_tiles_h3 = [
                        pvp.tile([P, 512], f32, tag="pv", name=f"pv3_{b}")
                        for b in range(3)
                    ]
                if idx2 <= 9:
                    emit_wo_use(0)
                # PV chase: s0 groups during s0 phase, s1 during s1
                if 1 <= idx2 <= 15:
                    tm = idx2 - 1
                    for g in range(8):
                        pv_mm(3, tm, g, pv_tiles_h3,
                              {(tm, 0): U[(3, tm, 0)]}, flags3)
                if idx2 == 16:
                    tm = 15
                    for g in range(8):
                        pv_mm(3, tm, g, pv_tiles_h3,
                              {(tm, 0): U[(3, tm, 0)]}, flags3)
                    # bank A (s-tiles 0..5) is complete: norm + queue flush
                    emit_norm(3, pv_tiles_h3, banks=(0,))
                if 17 <= idx2 <= 31:
                    tm = idx2 - 17
                    for g in range(8, 16):
                        pv_mm(3, tm, g, pv_tiles_h3,
                              {(tm, 1): U[(3, tm, 1)]}, flags3)
                # early flush of m=0..5 during the s1 phase
                if 18 <= idx2 <= 23:
                    m = idx2 - 18
                    emit_transpose(1, m, pv_tiles_h3[0])
                if 20 <= idx2 <= 31:
                    u = idx2 - 20          # 12 wop uses: (m, dj)
                    m, dj = u // 2, u % 2
                    wt = wop.tile([P, 512], f32, tag="wo",
                                  name=f"wo1e_{m}_{dj}")
                    nc.tensor.matmul(wt[:], OT2[1][m][:],
                                     wo_sb[1][:, dj * 512:(dj + 1) * 512],
                                     start=True, stop=True)
                    if m not in stage:
                        stage[m] = outp.tile([P, 1024], bf, tag="stg",
                                             name=f"stg1_{m}")
                    copy_out(stage[m][:, dj * 512:(dj + 1) * 512], wt[:],
                             eng="dve")
                    if dj == 1:
                        nc.sync.dma_start(
                            out=out_d[1, m * P:(m + 1) * P, :],
                            in_=stage[m][:])
                        del stage[m]
                idx2 += 1

            # ------------------------------------------------------------
            # Tail: last PV chunk, norm banks B/C, transposes m6..15,
            # Wo pair-1 for m=6..15 through the sc ring
            # ------------------------------------------------------------
            tm = 15
            for g in range(8, 16):
                pv_mm(3, tm, g, pv_tiles_h3,
                      {(tm, 1): U[(3, tm, 1)]}, flags3)
            emit_norm(3, pv_tiles_h3, banks=(1, 2))
            for m in range(6, NST):
                emit_transpose(1, m, pv_tiles_h3[1 + (m % 2)])

            tail_rr = [0]
            for m in range(6, NST):
                wt = scp.tile([P, 1024], f32, tag="sc", name=f"wo1_{m}")
                for dj in range(2):
                    nc.tensor.matmul(
                        wt[:, dj * 512:(dj + 1) * 512], OT2[1][m][:],
                        wo_sb[1][:, dj * 512:(dj + 1) * 512],
                        start=True, stop=True)
                stg = outp.tile([P, 1024], bf, tag="stg", name=f"stg1t_{m}")
                eng = ("scalar", "dve")[tail_rr[0] % 2]
                tail_rr[0] += 1
                copy_out(stg[:], wt[:], eng=eng)
                nc.sync.dma_start(out=out_d[1, m * P:(m + 1) * P, :],
                                  in_=stg[:])

    nc.finalize()
    return nc


def _prep_core_inputs(query, key, value, Wq, bq, Wk, bk, Wv, bv, Wo,
                      b, g, NH, DK):
    """Host-side shard prep for core (b, g): transpose+cast, pack weights."""
    D = query.shape[2]
    h0 = g * NH
    sl = slice(h0, h0 + NH)
    Wq_g, Wk_g, Wv_g = Wq[sl], Wk[sl], Wv[sl]
    NPAIR = NH // 2
    KT = D // P

    def pack_pair(W):
        # [NPAIR, 128, D]: pair p cols = heads (2p, 2p+1) concat; k-major free
        w = np.concatenate(
            [np.concatenate([W[2 * p], W[2 * p + 1]], axis=1)[None]
             for p in range(NPAIR)], axis=0)          # [NPAIR, D, 128]
        w = w.reshape(NPAIR, KT, P, P).transpose(0, 2, 1, 3).reshape(
            NPAIR, P, D)
        return w.astype(BF16)

    wq2 = pack_pair(Wq_g)
    wk2 = pack_pair(Wk_g)
    wv = np.concatenate([Wv_g[i] for i in range(NH)], axis=1)  # [D, NH*DK]
    NV = NH * DK
    wv4 = wv.reshape(KT, P, NV).transpose(1, 0, 2).reshape(
        P, KT * NV).astype(BF16)
    wo2 = Wo[h0 * DK:(h0 + NH) * DK].reshape(NPAIR, P, D).astype(BF16)
    return {
        "xqT": np.ascontiguousarray(query[b].T).astype(BF16),
        "xkT": np.ascontiguousarray(key[b].T).astype(BF16),
        "xvT": np.ascontiguousarray(value[b].T).astype(BF16),
        "wq2": wq2,
        "wk2": wk2,
        "wv4": wv4,
        "wo2": wo2,
    }


def kernel(query, key, value, Wq, bq, Wk, bk, Wv, bv, Wo, bo, _trace=False):
    from concourse.bass_utils import run_bass_kernel_spmd

    query = np.asarray(query, np.float32)
    key = np.asarray(key, np.float32)
    value = np.asarray(value, np.float32)
    B, S, D = query.shape
    H, _, DK = np.asarray(Wq).shape
    NCORE = 8
    GROUPS = NCORE // B
    NH = H // GROUPS

    # biases are all zero in this problem; verify and fold bo on host
    assert not (np.any(np.asarray(bq)) or np.any(np.asarray(bk))
                or np.any(np.asarray(bv))), "nonzero qkv bias unsupported"

    ck = ("nc",)
    if ck not in _CACHE:
        _CACHE[ck] = _build_nc(S, D, DK, NH)
    nc = _CACHE[ck]

    in_maps = []
    for c in range(NCORE):
        b, g = c // GROUPS, c % GROUPS
        in_maps.append(_prep_core_inputs(
            np.asarray(query), np.asarray(key), np.asarray(value),
            np.asarray(Wq), np.asarray(bq), np.asarray(Wk), np.asarray(bk),
            np.asarray(Wv), np.asarray(bv), np.asarray(Wo), b, g, NH, DK))

    res = run_bass_kernel_spmd(nc, in_maps, list(range(NCORE)), trace=_trace)
    out = np.zeros((B, S, D), np.float32)
    for c in range(NCORE):
        partials = np.asarray(res.results[c]["out"], np.float32)
        out[c // GROUPS] += partials[0] + partials[1]
    out += np.asarray(bo, np.float32)[None, None, :]
    if _trace:
        _CACHE["last_results"] = res
    return out
